# revision 39
# baseline (speedup 1.0000x reference)
"""Trainium2 Bass kernel for log-softmax multi-head attention (8 NeuronCores).

Reference computation (per batch):
    qkv = x @ w_qkv ; q,k,v per head
    dots = scale * q @ k^T ; attn = log_softmax(dots)
    out = attn @ v  -> merge heads -> out @ w_out + b_out + x

Algebraic identities used:
  1) log_softmax is linear in scores minus a row constant:
       attn = scale*dots - lse,  lse_i = ln sum_j exp(scale*dots_ij)
     so  out_head = scale * q @ (k^T v) - lse (x) colsum(v)
  2) k^T v = Wk^T (x^T x) Wv  (Gram matrix G = x^T x shared by all heads)
  3) colsum(v) = colsum(x) @ Wv
  4) the lse rank-1 correction commutes with the output projection:
       Y -= lnS_h (x) (vsum_h @ Wo_h)   summed over heads
  so the only O(n^2) work is the score matmul + exp/row-sum pass.

Sharding: 8 cores = 2 batches x 4 query-quarters. Every core computes k^T /
G for its full batch and q / lse / output for its own 1024 query rows ->
outputs disjoint, no collectives.

Schedule: the exp+rowsum pass on ScalarE is the hard floor (~265us at
FD=2048: 128 ACTIVATEs over [128,2048] PSUM tiles; accumulator reads overlap
the stream).  FD=2048 double-buffered uses all 8 PSUM banks, so auxiliary
matmul work (kT = Wk^T x^T, the Gram chain, OT precompute, base projections)
"rides" inside the dots buffers: each slot's tile is dead between its
accumulator read and its next fill, so a ride writes dtile[:, 0:512], a DVE
copy drains it, and the dots matmuls overwrite it.  Tile's dependency
tracker orders exp(i-2) -> ride MMs -> ride copy -> dots MMs -> exp(i).
Ln + the rank-1 correction run in a short tail (Exp and Ln live in
different ScalarE table sets; mixing them mid-stream thrashes table loads).
"""

import numpy as np

B, N, D = 2, 4096, 512
H, DH = 8, 64
SCALE = DH**-0.5
NQ = N // 4        # own query rows per core
QT = NQ // 128     # 8 own row tiles
NXT = N // 128     # 32 x row tiles

_GRAPH_CACHE = {}


def _build_graph():
    import concourse.bass as bass
    import concourse.tile as tile
    from concourse import bacc, mybir
    from concourse.masks import make_identity

    f32 = mybir.dt.float32
    bf16 = mybir.dt.bfloat16
    AF = mybir.ActivationFunctionType

    nc = bacc.Bacc("TRN2", target_bir_lowering=False, debug=False)

    xbf_d = nc.dram_tensor("x_bf", [N, D], bf16, kind="ExternalInput").ap()
    xq_d = nc.dram_tensor("xq", [NQ, D], f32, kind="ExternalInput").ap()
    wqkv_d = nc.dram_tensor("w_qkv_bf", [D, 3 * D], bf16, kind="ExternalInput").ap()
    wout_d = nc.dram_tensor("w_out_bf", [D, D], bf16, kind="ExternalInput").ap()
    bout_d = nc.dram_tensor("b_out", [D], f32, kind="ExternalInput").ap()
    out_d = nc.dram_tensor("out", [NQ, D], f32, kind="ExternalOutput").ap()

    with tile.TileContext(nc) as tc:
        with (
            tc.tile_pool(name="const", bufs=1) as const,
            tc.tile_pool(name="bigsb", bufs=1) as bigsb,
            tc.tile_pool(name="dout", bufs=2) as dout,
        ):
            # ------- constants + DMAs (issue order = priority) --------------
            ident_bf = const.tile([128, 128], bf16, tag="ident_bf")
            make_identity(nc, ident_bf[:])
            b_bc = const.tile([128, D], f32, tag="b_bc")
            nc.sync.dma_start(
                out=b_bc[:],
                in_=bass.AP(
                    tensor=bout_d.tensor,
                    offset=bout_d.offset,
                    ap=[[0, 128]] + [list(p) for p in bout_d.ap],
                ),
            )
            wq = []
            for j in range(4):
                w_t = const.tile([128, 3 * D], bf16, tag=f"wq{j}")
                nc.sync.dma_start(out=w_t[:], in_=wqkv_d[j * 128 : (j + 1) * 128, :])
                wq.append(w_t)

            xT = [bigsb.tile([128, N], bf16, name=f"xT{j}", tag=f"xT{j}") for j in range(4)]
            for r in range(8):
                for j in range(4):
                    nc.sync.dma_start(
                        out=xT[j][:, r * 512 : (r + 1) * 512],
                        in_=xbf_d[r * 512 : (r + 1) * 512, j * 128 : (j + 1) * 128],
                        transpose=True,
                    )
            wo = []
            for j in range(4):
                w_t = const.tile([128, D], bf16, tag=f"wo{j}")
                nc.sync.dma_start(out=w_t[:], in_=wout_d[j * 128 : (j + 1) * 128, :])
                wo.append(w_t)
            # residual rows (f32) -> become x + b via DVE adds under stream
            xb = []
            for t in range(QT):
                xb_t = dout.tile([128, D], f32, tag=f"xb{t}", bufs=1)
                nc.sync.dma_start(out=xb_t[:], in_=xq_d[t * 128 : (t + 1) * 128, :])
                xb.append(xb_t)
            # x row tiles (Gram matrix; consumed only after the stream, so
            # these DMAs are issued last and land during the exp stream)
            xrow = []
            for t in range(NXT):
                xr_t = bigsb.tile([128, D], bf16, tag=f"xrow{t}")
                nc.sync.dma_start(out=xr_t[:], in_=xbf_d[t * 128 : (t + 1) * 128, :])
                xrow.append(xr_t)

            # ------- big SBUF operands --------------------------------------
            qT = [bigsb.tile([128, NQ], bf16, name=f"qT{c}", tag=f"qT{c}") for c in range(4)]
            kT = [bigsb.tile([128, N], bf16, name=f"kT{c}", tag=f"kT{c}") for c in range(4)]
            G_sb = [bigsb.tile([128, D], f32, name=f"G{j}", tag=f"G{j}") for j in range(4)]
            G_bf = [bigsb.tile([128, D], bf16, name=f"Gb{j}", tag=f"Gb{j}") for j in range(4)]
            GWk = [bigsb.tile([128, D], bf16, name=f"GWk{j}", tag=f"GWk{j}") for j in range(4)]
            KVW = [bigsb.tile([128, D], bf16, name=f"KVW{c}", tag=f"KVW{c}") for c in range(4)]
            Yb = [bigsb.tile([128, D], f32, name=f"Yb{t}", tag=f"Yb{t}") for t in range(QT)]
            kv_p = const.tile([128, D], bf16, tag="kv_p")
            nc.vector.memset(kv_p[:], 0.0)
            csx4 = [const.tile([128, 4], f32, name=f"csx4_{j}", tag=f"csx4_{j}") for j in range(4)]
            csx_bf = [const.tile([128, 1], bf16, name=f"csxb{j}", tag=f"csxb{j}") for j in range(4)]
            vsT = [const.tile([128, 1], bf16, name=f"vsT{j}", tag=f"vsT{j}") for j in range(4)]
            VSmat = [const.tile([128, 8], bf16, name=f"VSm{j}", tag=f"VSm{j}") for j in range(4)]
            for j in range(4):
                nc.vector.memset(VSmat[j][:], 0.0)
            W8_sb = const.tile([8, D], bf16, tag="W8")
            lse_acc = const.tile([128, 128], f32, tag="lse_acc")
            lse_sum = const.tile([128, 64], f32, tag="lse_sum")
            lse_ln = const.tile([128, 64], bf16, tag="lse_ln")
            lnST = const.tile([8, NQ], bf16, tag="lnST")
            dummy = const.tile([128, 1], f32, tag="dummy")
            nc.vector.memset(dummy[:], 0.0)

            # preload the Exp table set before the stream
            nc.scalar.activation(out=dummy[:], in_=dummy[:], func=AF.Exp)
            if True:

                # ---- ride bodies: scr is a [128,512] f32 psum view ---------
                cp_flip = [0]

                def any_copy(dst, srcv):
                    # alternate ScalarE/DVE so neither copy queue backlogs
                    cp_flip[0] ^= 1
                    if cp_flip[0]:
                        nc.scalar.copy(dst, srcv)
                    else:
                        nc.vector.tensor_copy(dst, srcv)

                def qT_half(scr, c, nn):
                    for j in range(4):
                        nc.tensor.matmul(
                            scr,
                            lhsT=wq[j][:, c * 128 : (c + 1) * 128],
                            rhs=xT[j][:, nn * 512 : (nn + 1) * 512],
                            start=(j == 0),
                            stop=(j == 3),
                        )
                    any_copy(qT[c][:, nn * 512 : (nn + 1) * 512], scr)

                def kT_chunk(scr, c, ch):
                    for j in range(4):
                        nc.tensor.matmul(
                            scr,
                            lhsT=wq[j][:, 512 + c * 128 : 512 + (c + 1) * 128],
                            rhs=xT[j][:, ch * 512 : (ch + 1) * 512],
                            start=(j == 0),
                            stop=(j == 3),
                        )
                    any_copy(kT[c][:, ch * 512 : (ch + 1) * 512], scr)

                def g_chunk(scr, jm, t0):
                    for t in range(t0, t0 + 4):
                        nc.tensor.matmul(
                            scr,
                            lhsT=xrow[t][:, jm * 128 : (jm + 1) * 128],
                            rhs=xrow[t][:],
                            start=(t == t0),
                            stop=(t == t0 + 3),
                        )
                    if t0 == 0:
                        nc.vector.tensor_copy(G_sb[jm][:], scr)
                    else:
                        nc.vector.tensor_add(G_sb[jm][:], G_sb[jm][:], scr)

                def g_fin(jm):
                    any_copy(G_bf[jm][:], G_sb[jm][:])

                def gwk_jm(scr, jm):
                    # GWk[jm] = (G @ Wk) rows jm*128:(jm+1)*128
                    for j in range(4):
                        nc.tensor.matmul(
                            scr,
                            lhsT=G_bf[j][:, jm * 128 : (jm + 1) * 128],
                            rhs=wq[j][:, 512:1024],
                            start=(j == 0),
                            stop=(j == 3),
                        )
                    any_copy(GWk[jm][:], scr)

                def kvt_head(scr, h):
                    # kv^T_h = Wv_h^T (G Wk)_h ; scaled into kv_p rows r0
                    r0 = (h % 2) * 64
                    for j in range(4):
                        nc.tensor.matmul(
                            scr[0:64, 0:64],
                            lhsT=wq[j][:, 1024 + h * 64 : 1024 + (h + 1) * 64],
                            rhs=GWk[j][:, h * 64 : (h + 1) * 64],
                            start=(j == 0),
                            stop=(j == 3),
                        )
                    nc.vector.tensor_scalar_mul(
                        kv_p[r0 : r0 + 64, h * 64 : (h + 1) * 64], scr[0:64, 0:64], SCALE
                    )

                def kvw_c(scr, c):
                    # KVW_c[b-rows, :] = scale * kv_h @ Wo_h for both heads of c
                    for hp in range(2):
                        h, r0 = 2 * c + hp, hp * 64
                        nc.tensor.matmul(
                            scr[r0 : r0 + 64, :],
                            lhsT=kv_p[:, h * 64 : (h + 1) * 64],
                            rhs=wo[c][:],
                            start=True,
                            stop=True,
                        )
                    any_copy(KVW[c][:], scr)

                def csx_piece(scr, j, p):
                    nc.vector.tensor_reduce(
                        csx4[j][:, p : p + 1],
                        xT[j][:, p * 1024 : (p + 1) * 1024],
                        axis=mybir.AxisListType.X,
                        op=mybir.AluOpType.add,
                    )

                def csx_fin(scr, j):
                    nc.vector.tensor_reduce(
                        csx4[j][:, 0:1], csx4[j][:],
                        axis=mybir.AxisListType.X, op=mybir.AluOpType.add,
                    )
                    nc.vector.tensor_copy(csx_bf[j][:], csx4[j][:, 0:1])

                def vsum_jm(scr, jm):
                    # vsT[jm] = -(Wv^T colsum(x)) block jm (minus sign -> W8)
                    for j in range(4):
                        nc.tensor.matmul(
                            scr[:, 0:1],
                            lhsT=wq[j][:, 1024 + jm * 128 : 1024 + (jm + 1) * 128],
                            rhs=csx_bf[j][:],
                            start=(j == 0),
                            stop=(j == 3),
                        )
                    nc.vector.tensor_scalar_mul(vsT[jm][:], scr[:, 0:1], -1.0)
                    nc.vector.tensor_copy(
                        VSmat[jm][0:64, 2 * jm : 2 * jm + 1], vsT[jm][0:64, :]
                    )
                    nc.vector.tensor_copy(
                        VSmat[jm][64:128, 2 * jm + 1 : 2 * jm + 2], vsT[jm][64:128, :]
                    )

                def w8_mm(scr):
                    for j in range(4):
                        nc.tensor.matmul(
                            scr[0:8, :],
                            lhsT=VSmat[j][:],
                            rhs=wo[j][:],
                            start=(j == 0),
                            stop=(j == 3),
                        )
                    nc.vector.tensor_copy(W8_sb[:], scr[0:8, :])

                def ybase(scr, t):
                    for c in range(4):
                        nc.tensor.matmul(
                            scr,
                            lhsT=qT[c][:, t * 128 : (t + 1) * 128],
                            rhs=KVW[c][:],
                            start=(c == 0),
                            stop=(c == 3),
                        )
                    nc.vector.tensor_add(Yb[t][:], scr, xb[t][:])

                def xb_add(scr, t):
                    nc.vector.tensor_add(xb[t][:], xb[t][:], b_bc[:])

                # ---- ride schedule (FIFO; one MM ride per stream slot,
                # DVE-only rides drain from their own queue) -----------------
                head_rides = []   # feed the exp stream (kT / qT)
                tail_rides = []   # feed only the output tail (Gram chain)
                dve_rides = []
                for c in range(1, 4):                     # qT: needs only
                    for nn in range(2):                   # xT cols 0:1024
                        head_rides.append(lambda s, c=c, nn=nn: qT_half(s, c, nn))
                for ch in range(4):                       # kT in DMA order
                    for c in range(4):
                        if c == 0 and ch < 4:
                            pass  # kT[0] ch0-3 handled below too; skip dups
                for ch in range(4):
                    for c in range(1, 4):
                        head_rides.append(lambda s, c=c, ch=ch: kT_chunk(s, c, ch))
                for ch in range(4, 8):
                    for c in range(4):
                        head_rides.append(lambda s, c=c, ch=ch: kT_chunk(s, c, ch))
                for jm in range(4):                       # Gram matrix
                    for t0 in range(0, NXT, 4):
                        tail_rides.append(lambda s, jm=jm, t0=t0: g_chunk(s, jm, t0))
                    tail_rides.append(lambda s, jm=jm: g_fin(jm))
                for j in range(4):
                    for p in range(4):
                        dve_rides.append(lambda j=j, p=p: csx_piece(None, j, p))
                for j in range(4):
                    dve_rides.append(lambda j=j: csx_fin(None, j))
                for t in range(QT):
                    dve_rides.append(lambda t=t: xb_add(None, t))
                for jm in range(4):                       # GWk (bf16)
                    tail_rides.append(lambda s, jm=jm: gwk_jm(s, jm))
                for h in range(H):
                    tail_rides.append(lambda s, h=h: kvt_head(s, h))
                for jm in range(4):
                    tail_rides.append(lambda s, jm=jm: vsum_jm(s, jm))
                tail_rides.append(lambda s: w8_mm(s))
                for c in range(4):
                    tail_rides.append(lambda s, c=c: kvw_c(s, c))
                for t in range(QT):
                    tail_rides.append(lambda s, t=t: ybase(s, t))

            # ---- head: all aux units on a deep-buffered scratch pool ------
            with tc.tile_pool(name="head_ps", bufs=1, space="PSUM") as hps:
                def head_tile():
                    return hps.tile([128, 512], f32, name="hsc", tag="hsc", bufs=8)

                for nn in range(2):
                    qT_half(head_tile()[:], 0, nn)
                for ch in range(4):
                    kT_chunk(head_tile()[:], 0, ch)
                while head_rides:
                    head_rides.pop(0)(head_tile()[:])

            # ---- the exp stream: 128 slots of FD=2048 ----------------------
            with tc.tile_pool(name="dots_ps", bufs=1, space="PSUM") as dps:
                def new_tile():
                    return dps.tile([128, 2048], f32, name="dots", tag="dots", bufs=2)
                for h in range(H):
                    c, r0 = h // 2, (h % 2) * 64
                    for t in range(QT):
                        lhsT = qT[c][r0 : r0 + 64, t * 128 : (t + 1) * 128]
                        for half in range(2):
                            dtile = new_tile()
                            for cc in range(4):
                                nc.tensor.matmul(
                                    dtile[:, cc * 512 : (cc + 1) * 512],
                                    lhsT=lhsT,
                                    rhs=kT[c][
                                        r0 : r0 + 64,
                                        (half * 4 + cc) * 512 : (half * 4 + cc + 1) * 512,
                                    ],
                                    start=True,
                                    stop=True,
                                )
                            col = (h * 8 + t) * 2 + half
                            nc.scalar.activation(
                                out=dtile[:],
                                in_=dtile[:],
                                func=AF.Exp,
                                scale=SCALE,
                                accum_out=lse_acc[:, col : col + 1],
                            )

            # ---- tail: Gram-chain aux (deep-buffered) overlapped with the
            # lse -> Ln scalar work, then rank-1 + residual ------------------
            with tc.tile_pool(name="tail_ps", bufs=1, space="PSUM") as tps:
                def tail_tile():
                    return tps.tile([128, 512], f32, name="tsc", tag="tsc", bufs=8)

                # scalar/DVE lse work first: overlaps the PE aux below
                la = lse_acc[:].rearrange("q (p two) -> q p two", two=2)
                nc.vector.tensor_add(lse_sum[:], la[:, :, 0], la[:, :, 1])
                nc.scalar.activation(out=lse_ln[:], in_=lse_sum[:], func=AF.Ln)
                # lse_ln cols are h*8+t; gather per-t slices into t-major
                lse_tm = const.tile([128, 64], bf16, tag="lse_tm")
                nc.vector.tensor_copy(
                    lse_tm[:],
                    lse_ln[:].rearrange("q (h t) -> q t h", t=QT),
                )
                nu = 0
                while tail_rides:
                    tail_rides.pop(0)(tail_tile()[:])
                    nu += 1
                    if dve_rides and nu % 2 == 0:
                        dve_rides.pop(0)()
                while dve_rides:
                    dve_rides.pop(0)()
                for t in range(QT):
                    ps = tail_tile()
                    ps_bf = ps[0:8, 0:64].bitcast(bf16)
                    nc.tensor.transpose(ps_bf, lse_tm[:, t * 8 : (t + 1) * 8], ident_bf[:])
                    nc.vector.tensor_copy(lnST[:, t * 128 : (t + 1) * 128], ps_bf)
                    yps = tail_tile()
                    nc.tensor.matmul(
                        yps[:, 0:512],
                        lhsT=lnST[:, t * 128 : (t + 1) * 128],
                        rhs=W8_sb[:],
                        start=True,
                        stop=True,
                    )
                    ysb = dout.tile([128, D], f32, name="ysb", tag="ysb")
                    nc.vector.tensor_add(ysb[:], yps[:, 0:512], Yb[t][:])
                    nc.sync.dma_start(out=out_d[t * 128 : (t + 1) * 128, :], in_=ysb[:])

    nc.compile()
    return nc


def get_graph():
    if "nc" not in _GRAPH_CACHE:
        _GRAPH_CACHE["nc"] = _build_graph()
    return _GRAPH_CACHE["nc"]


def make_in_maps(x, w_qkv, w_out, b_out):
    import ml_dtypes

    x = np.ascontiguousarray(x, dtype=np.float32)
    w_qkv = np.ascontiguousarray(w_qkv, dtype=np.float32)
    w_out = np.ascontiguousarray(w_out, dtype=np.float32)
    b_out = np.ascontiguousarray(b_out, dtype=np.float32)
    x_bf = x.astype(ml_dtypes.bfloat16)
    w_qkv_bf = w_qkv.astype(ml_dtypes.bfloat16)
    w_out_bf = w_out.astype(ml_dtypes.bfloat16)
    in_maps = []
    for i in range(8):
        b, q = divmod(i, 4)
        in_maps.append(
            {
                # keys are permutation-invariant for lse/kv/G; roll so this
                # core's own query rows sit at rows 0:NQ
                "x_bf": np.ascontiguousarray(np.roll(x_bf[b], -q * NQ, axis=0)),
                "xq": np.ascontiguousarray(x[b, q * NQ : (q + 1) * NQ]),
                "w_qkv_bf": w_qkv_bf,
                "w_out_bf": w_out_bf,
                "b_out": b_out,
            }
        )
    return in_maps


def kernel(x, w_qkv, w_out, b_out):
    from concourse.bass_utils import run_bass_kernel_spmd

    nc = get_graph()
    in_maps = make_in_maps(x, w_qkv, w_out, b_out)
    res = run_bass_kernel_spmd(nc, in_maps, core_ids=list(range(8)))
    out = np.empty((B, N, D), np.float32)
    for i in range(8):
        b, q = divmod(i, 4)
        out[b, q * NQ : (q + 1) * NQ] = res.results[i]["out"]
    return out
